# revision 1
# baseline (speedup 1.0000x reference)
"""Trainium2 Bass kernel for a dense transformer block (nn_Block_15058155340426).

Full (unsharded) inputs -> full output.  Internally: 8-core SPMD, no
cross-core communication.  Core c handles batch b=c//2, token half c%2.
Each core redundantly computes LN1 + K/V for its batch's full 2048 tokens
(order: [own 1024 | other 1024]) and runs attention/proj/MLP for its own
1024 query tokens.  All on-chip activations use the transposed [feature,
token] layout; GEMM operands are bf16; the residual stream stays fp32.

Perf structure: the K/Q projection GEMMs are interleaved into the
attention stream as PE filler (attention alone is ACT/exp-bound, which
lets the PE idle and the HAM clock-gate throttle it to 1.2 GHz).
"""

import numpy as np
import ml_dtypes

import concourse.bass as bass
import concourse.bacc as bacc
import concourse.tile as tile
from concourse import mybir
from concourse.bass_utils import run_bass_kernel_spmd

BF = mybir.dt.bfloat16
F32 = mybir.dt.float32
BF_NP = ml_dtypes.bfloat16

DIM = 768
HEADS = 12
HD = 64          # head dim
HID = 3072
B = 4
N = 2048         # tokens per batch (full)
NO = 1024        # own tokens per core
CC = DIM // 128  # 6 feature chunks
HC = HID // 128  # 24 hidden chunks
MC = N // 128    # 16 key chunks
EPS = 1e-5
SCALE = HD ** -0.5

AF = mybir.ActivationFunctionType

_compiled = [None]


def _build_nc():
    nc = bacc.Bacc("TRN2", target_bir_lowering=False, debug=False, num_devices=8)

    # ---- DRAM I/O ----
    dt_ = {}
    dt_["xTb"] = nc.dram_tensor("xTb", [DIM, N], BF, kind="ExternalInput")
    dt_["xTf"] = nc.dram_tensor("xTf", [DIM, NO], F32, kind="ExternalInput")
    dt_["wqkv"] = nc.dram_tensor("wqkv", [DIM, 3 * DIM], BF, kind="ExternalInput")
    dt_["wproj"] = nc.dram_tensor("wproj", [DIM, DIM], BF, kind="ExternalInput")
    dt_["wfc1"] = nc.dram_tensor("wfc1", [DIM, HID], BF, kind="ExternalInput")
    dt_["wfc2"] = nc.dram_tensor("wfc2", [HID, DIM], BF, kind="ExternalInput")
    dt_["gb1"] = nc.dram_tensor("gb1", [DIM, 2], F32, kind="ExternalInput")
    dt_["gb2"] = nc.dram_tensor("gb2", [DIM, 2], F32, kind="ExternalInput")
    dt_["bproj"] = nc.dram_tensor("bproj", [DIM, 1], F32, kind="ExternalInput")
    dt_["bfc1"] = nc.dram_tensor("bfc1", [HID, 1], F32, kind="ExternalInput")
    dt_["bfc2"] = nc.dram_tensor("bfc2", [DIM, 1], F32, kind="ExternalInput")
    dt_["ones"] = nc.dram_tensor("ones", [128, 128], BF, kind="ExternalInput")
    dt_["outT"] = nc.dram_tensor("outT", [DIM, NO], F32, kind="ExternalOutput")

    with tile.TileContext(nc, pool_alloc_mode="queue") as tc:
        _emit(nc, tc, dt_)
    nc.compile()
    return nc


def _ln_rows(nc, rows, musum, sqsum, mu_bf, rs_bf, epst, n):
    """Row math on partition 0: mean/var -> bf16 mu and rsqrt rows."""
    mu_f = rows.tile([1, n], F32, tag="rowf", name="mu_f")
    nc.scalar.mul(mu_f, musum, 1.0 / DIM)
    nc.vector.tensor_copy(mu_bf, mu_f)
    var = rows.tile([1, n], F32, tag="rowf", name="var")
    nc.vector.tensor_mul(var, mu_f, mu_f)          # var := mu^2
    ex2 = rows.tile([1, n], F32, tag="rowf", name="ex2")
    nc.scalar.mul(ex2, sqsum, 1.0 / DIM)
    nc.vector.tensor_sub(var, ex2, var)            # var := E[x^2] - mu^2
    sd = rows.tile([1, n], F32, tag="rowf", name="sd")
    nc.scalar.activation(sd, var, AF.Sqrt, bias=epst[0:1, 0:1])
    rs_f = rows.tile([1, n], F32, tag="rowf", name="rs_f")
    nc.vector.reciprocal_approx_fast(rs_f, sd)
    nc.vector.tensor_copy(rs_bf, rs_f)


def _emit(nc, tc, dt_):
    from contextlib import ExitStack
    es = ExitStack()            # whole-kernel pools (released at end, LIFO)
    es_qkv = ExitStack()        # hT + wqkv: until attention done
    es_kqv = ExitStack()        # kTs/qTs/vt: until attention done
    es_att = ExitStack()        # attention working tiles
    with es:
        # ---------- pool allocation (stack order = reverse of release) ----------
        const = es.enter_context(tc.tile_pool(name="const", bufs=1))
        ones_sb = const.tile([128, 128], BF)
        gb1_sb = const.tile([128, CC, 2], F32)
        gb2_sb = const.tile([128, CC, 2], F32)
        bproj_sb = const.tile([128, CC, 1], F32)
        bfc1_sb = const.tile([128, HC, 1], F32)
        bfc2_sb = const.tile([128, CC, 1], F32)
        epst = const.tile([1, 1], F32)

        # long-lived: residual (becomes x2T in place), attention output, wproj
        resid = es.enter_context(tc.tile_pool(name="resid", bufs=1))
        xTf = resid.tile([128, CC, NO], F32)
        x2T = xTf  # alias: P4 writes the residual sum back into this tile
        oTpair = resid.tile([128, CC, NO], BF)
        wproj_sb = resid.tile([128, CC, DIM], BF)

        pool_h = es_qkv.enter_context(tc.tile_pool(name="h", bufs=1))
        hT = pool_h.tile([128, CC, N], BF)
        pool_wqkv = es_qkv.enter_context(tc.tile_pool(name="wqkv", bufs=1))
        wqkv_sb = pool_wqkv.tile([128, CC, 3 * DIM], BF)

        pool_kqv = es_kqv.enter_context(tc.tile_pool(name="kqv", bufs=1))
        kTs = pool_kqv.tile([128, CC, N], BF)       # head pairs: part 0:64 = head 2hp
        qTs = pool_kqv.tile([128, CC, NO], BF)
        vt = pool_kqv.tile([128, MC, HEADS, HD + 1], BF)

        # ---------- Phase 1+2: LN1 (two pipelined halves) + V GEMMs ----------
        with tc.tile_pool(name="ln1", bufs=1) as ln1, \
             tc.tile_pool(name="rows1", bufs=2) as rows1, \
             tc.tile_pool(name="ln1sq", bufs=2) as ln1sq, \
             tc.tile_pool(name="ln1tmp", bufs=2) as ln1tmp, \
             tc.tile_pool(name="ps_ln1", bufs=1, space="PSUM") as psL:
            xTb = ln1.tile([128, CC, N], BF)
            # DMA priority: xTb (gates everything) first, then consts, weights
            for cc in range(CC):
                nc.sync.dma_start(
                    out=xTb[:, cc, :],
                    in_=dt_["xTb"].ap().rearrange("(t p) n -> p t n", p=128)[:, cc, :])
            nc.sync.dma_start(out=ones_sb[:], in_=dt_["ones"][:])
            nc.sync.dma_start(out=gb1_sb[:], in_=dt_["gb1"].ap().rearrange("(t p) k -> p t k", p=128))
            nc.sync.dma_start(out=gb2_sb[:], in_=dt_["gb2"].ap().rearrange("(t p) k -> p t k", p=128))
            nc.sync.dma_start(out=bproj_sb[:], in_=dt_["bproj"].ap().rearrange("(t p) k -> p t k", p=128))
            nc.sync.dma_start(out=bfc1_sb[:], in_=dt_["bfc1"].ap().rearrange("(t p) k -> p t k", p=128))
            nc.sync.dma_start(out=bfc2_sb[:], in_=dt_["bfc2"].ap().rearrange("(t p) k -> p t k", p=128))
            nc.vector.memset(epst, EPS)
            nc.sync.dma_start(out=wqkv_sb[:], in_=dt_["wqkv"].ap().rearrange("(t p) m -> p t m", p=128))
            nc.vector.memset(vt, 1.0)

            for half in range(2):
                nsl = slice(half * NO, (half + 1) * NO)
                st = psL.tile([1, 2 * NO], F32, tag="st", bufs=1, name="st")
                for cc in range(CC):
                    sq = ln1sq.tile([128, NO], BF, tag="sq")
                    nc.vector.tensor_mul(sq, xTb[:, cc, nsl], xTb[:, cc, nsl])
                    for s in range(NO // 512):
                        isl = slice(s * 512, (s + 1) * 512)
                        xsl = slice(half * NO + s * 512, half * NO + (s + 1) * 512)
                        nc.tensor.matmul(st[:, isl], ones_sb[:, 0:1], xTb[:, cc, xsl],
                                         start=(cc == 0), stop=(cc == CC - 1))
                        nc.tensor.matmul(st[:, NO + s * 512: NO + (s + 1) * 512],
                                         ones_sb[:, 0:1], sq[:, isl],
                                         start=(cc == 0), stop=(cc == CC - 1))
                mu_bf = rows1.tile([1, NO], BF, tag="mubf", bufs=2, name="mu_bf")
                rs_bf = rows1.tile([1, NO], BF, tag="rsbf", bufs=2, name="rs_bf")
                _ln_rows(nc, rows1, st[:, 0:NO], st[:, NO:2 * NO], mu_bf, rs_bf,
                         epst, NO)
                muB = psL.tile([128, NO], F32, tag="bc", bufs=2, name="muB")
                rsB = psL.tile([128, NO], F32, tag="bc", bufs=2, name="rsB")
                for s in range(NO // 512):
                    isl = slice(s * 512, (s + 1) * 512)
                    nc.tensor.matmul(muB[:, isl], ones_sb[0:1, :], mu_bf[:, isl])
                    nc.tensor.matmul(rsB[:, isl], ones_sb[0:1, :], rs_bf[:, isl])
                for cc in range(CC):
                    t1 = ln1tmp.tile([128, NO], BF, tag="t", name="t1")
                    nc.vector.tensor_sub(t1, xTb[:, cc, nsl], muB)
                    t2 = ln1tmp.tile([128, NO], BF, tag="t", name="t2")
                    nc.vector.tensor_mul(t2, t1, rsB)
                    nc.scalar.activation(hT[:, cc, nsl], t2, AF.Identity,
                                         bias=gb1_sb[:, cc, 1:2],
                                         scale=gb1_sb[:, cc, 0:1])
                # V GEMMs for this half's key chunks (PE filler behind LN DVE/ACT)
                for mi in range(half * (MC // 2), (half + 1) * (MC // 2)):
                    vps = psL.tile([128, DIM], F32, tag="bc", bufs=2, name="vps")
                    for cc in range(CC):
                        lhsT = hT[:, cc, mi * 128: (mi + 1) * 128]
                        nc.tensor.matmul(vps[:, 0:512], lhsT, wqkv_sb[:, cc, 1536:2048],
                                         start=(cc == 0), stop=(cc == CC - 1))
                        nc.tensor.matmul(vps[:, 512:768], lhsT, wqkv_sb[:, cc, 2048:2304],
                                         start=(cc == 0), stop=(cc == CC - 1))
                    nc.vector.tensor_copy(
                        vt[:, mi, :, 0:HD],
                        vps.rearrange("p (h d) -> p h d", h=HEADS))
                # K/Q for pair 0, this token-half (enables early attention)
                for col in range(half * 2, half * 2 + 2):
                    kps = psL.tile([128, 512], F32, tag="bc", bufs=2, name="kps")
                    for cc in range(CC):
                        nc.tensor.matmul(
                            kps[:, 0:512], wqkv_sb[:, cc, DIM: DIM + 128],
                            hT[:, cc, col * 512:(col + 1) * 512],
                            start=(cc == 0), stop=(cc == CC - 1))
                    nc.vector.tensor_copy(kTs[:, 0, col * 512:(col + 1) * 512],
                                          kps[:, 0:512])
                qps = psL.tile([128, 512], F32, tag="bc", bufs=2, name="qps")
                for cc in range(CC):
                    nc.tensor.matmul(
                        qps[:, 0:512], wqkv_sb[:, cc, 0:128],
                        hT[:, cc, half * 512:(half + 1) * 512],
                        start=(cc == 0), stop=(cc == CC - 1))
                nc.vector.tensor_copy(qTs[:, 0, half * 512:(half + 1) * 512],
                                      qps[:, 0:512])

        # ---------- Phase 3: attention with K/Q GEMMs interleaved ----------
        es_attps = ExitStack()
        psS = es_attps.enter_context(tc.tile_pool(name="ps_S", bufs=2, space="PSUM"))
        psO = es_attps.enter_context(tc.tile_pool(name="ps_oT", bufs=2, space="PSUM"))
        attw = es_att.enter_context(tc.tile_pool(name="attw", bufs=3))
        atodd = es_att.enter_context(tc.tile_pool(name="atodd", bufs=1))

        def kq_chunks(hp):
            """Six PE filler chunks producing kTs / qTs for pair hp; each is
            a closure so it can be spliced between attention m-steps."""
            out = []
            for half in range(4):   # k: 4 x 512 columns
                def k_grp(hp=hp, half=half):
                    kps = psS.tile([128, 512], F32, tag="S", name="kps")
                    for cc in range(CC):
                        nc.tensor.matmul(
                            kps[:], wqkv_sb[:, cc, DIM + hp * 128: DIM + (hp + 1) * 128],
                            hT[:, cc, half * 512:(half + 1) * 512],
                            start=(cc == 0), stop=(cc == CC - 1))
                    nc.vector.tensor_copy(kTs[:, hp, half * 512:(half + 1) * 512], kps)
                out.append(k_grp)
            for half in range(2):   # q: 2 x 512 columns (own tokens)
                def q_grp(hp=hp, half=half):
                    qps = psS.tile([128, 512], F32, tag="S", name="qps")
                    for cc in range(CC):
                        nc.tensor.matmul(
                            qps[:], wqkv_sb[:, cc, hp * 128: (hp + 1) * 128],
                            hT[:, cc, half * 512:(half + 1) * 512],
                            start=(cc == 0), stop=(cc == CC - 1))
                    nc.vector.tensor_copy(qTs[:, hp, half * 512:(half + 1) * 512], qps)
                out.append(q_grp)
            return out

        def stage(oTp):
            # one cast drains the whole unnormalized head (incl colsum row)
            # to SBUF so the PSUM banks free immediately for the next pair
            oTu = attw.tile([65, NO], F32, tag="oTu", bufs=2, name="oTu")
            nc.vector.tensor_copy(oTu, oTp)
            return oTu

        def normalize(hp, oTu0, oTu1):
            # full-tile fast reciprocal of the staged head (only the colsum
            # row 64 is meaningful; a row-slice approx_fast is broken) ->
            # broadcast row 64 to 64 partitions -> scale the head.
            for off, oTu in ((0, oTu0), (64, oTu1)):
                csr = attw.tile([65, NO], F32, tag="csrf", bufs=2, name="csr")
                nc.vector.reciprocal_approx_fast(csr, oTu)
                csrb = attw.tile([65, NO], BF, tag="csrb", bufs=2, name="csrb")
                nc.vector.tensor_copy(csrb[64:65, :], csr[64:65, :])
                rcpB = psS.tile([64, NO], F32, tag="S", name="rcpB")
                for s in range(NO // 512):
                    sl = slice(s * 512, (s + 1) * 512)
                    nc.tensor.matmul(rcpB[:, sl], ones_sb[64:65, 0:64],
                                     csrb[64:65, sl])
                rcpS = attw.tile([64, NO], BF, tag="rcps", bufs=2, name="rcpS")
                nc.vector.tensor_copy(rcpS, rcpB)
                if off == 0:
                    nc.vector.tensor_mul(oTpair[0:64, hp, :], oTu[0:64, :], rcpS)
                else:
                    stag = atodd.tile([64, NO], BF, tag="stag", name="stag")
                    nc.vector.tensor_mul(stag, oTu[0:64, :], rcpS)
                    nc.sync.dma_start(out=oTpair[64:128, hp, :], in_=stag)

        pending = None  # (hp, oTu0, oTu1) deferred so PE never waits on recip
        for hp in range(CC):
            h0, h1 = 2 * hp, 2 * hp + 1
            filler = kq_chunks(hp + 1) if hp + 1 < CC else []
            if hp == 3:
                # load late-phase inputs during attention (DMA engines idle)
                nc.sync.dma_start(out=wproj_sb[:],
                                  in_=dt_["wproj"].ap().rearrange("(t p) m -> p t m", p=128))
                nc.sync.dma_start(out=xTf[:],
                                  in_=dt_["xTf"].ap().rearrange("(t p) n -> p t n", p=128))
            oTp0 = psO.tile([65, NO], F32, tag="oT", name="oTp0")
            oTp1 = psO.tile([65, NO], F32, tag="oT", name="oTp1")
            deferred = []   # avs of mi==0 held back so the new pair opens
                            # with a dense run of S matmuls
            for mi in range(MC):
                ats = []
                for off in (0, 64):
                    S = psS.tile([128, NO], F32, tag="S", name="S")
                    for s in range(NO // 512):
                        sl = slice(s * 512, (s + 1) * 512)
                        nc.tensor.matmul(S[:, sl],
                                         kTs[off:off + 64, hp, mi * 128:(mi + 1) * 128],
                                         qTs[off:off + 64, hp, sl])
                    at = attw.tile([128, NO], BF, tag="at", name="at", bufs=4)
                    nc.scalar.activation(at, S, AF.Exp, scale=SCALE)
                    ats.append(at)
                # splice one K/Q filler chunk for the NEXT pair here: keeps
                # the PE busy while ACT computes the exps
                if mi in (1, 3, 6, 9, 12, 14) and filler:
                    filler.pop(0)()
                elif not filler and mi % 2 == 0:
                    # no real filler (last pair): dummy weight loads keep the
                    # PE array active so the HAM clock gate stays open
                    nc.tensor.ldweights(weights=vt[:, mi, 0, :])
                    nc.tensor.ldweights(weights=vt[:, mi, 6, :])
                work = [(h0, oTp0, ats[0]), (h1, oTp1, ats[1])]
                if mi == 0:
                    deferred = work
                    continue
                for h, oTp, at in (deferred + work if mi == 1 else work):
                    for s in range(NO // 512):
                        sl = slice(s * 512, (s + 1) * 512)
                        nc.tensor.matmul(oTp[:, sl], vt[:, mi if at in
                                         (ats[0], ats[1]) else 0, h, :], at[:, sl],
                                         start=(at not in (ats[0], ats[1])),
                                         stop=(mi == MC - 1))
                if mi == 1:
                    deferred = []
                if mi == 2 and pending is not None:
                    normalize(*pending)
                    pending = None
            for grp in filler:
                grp()
            oTu0, oTu1 = stage(oTp0), stage(oTp1)
            pending = (hp, oTu0, oTu1)
        normalize(*pending)
        es_att.close()
        es_kqv.close()
        es_qkv.close()
        psP = psS    # proj accumulators reuse the attention score slots
        ps_st2 = psO  # LN2 stats reuse the attention output slots

        # ---------- Phase 4: proj + residual (written in place into xTf) ----------
        pool_wfc1 = es.enter_context(tc.tile_pool(name="wfc1", bufs=1))
        wfc1_sb = pool_wfc1.tile([128, CC, HID], BF)
        for cc in range(CC):
            nc.sync.dma_start(
                out=wfc1_sb[:, cc, :],
                in_=dt_["wfc1"].ap().rearrange("(t p) m -> p t m", p=128)[:, cc, :])
        pool_wfc2 = es.enter_context(tc.tile_pool(name="wfc2", bufs=1))
        wfc2_sb = pool_wfc2.tile([128, HC, DIM], BF)
        for hc2 in range(0, HC, 6):
            nc.sync.dma_start(
                out=wfc2_sb[:, hc2:hc2 + 6, :],
                in_=dt_["wfc2"].ap().rearrange("(t p) m -> p t m", p=128)[:, hc2:hc2 + 6, :])
        pool_h2 = es.enter_context(tc.tile_pool(name="h2", bufs=1))
        h2T = pool_h2.tile([128, CC, NO], BF)
        ln2stack = ExitStack()
        rows2 = ln2stack.enter_context(tc.tile_pool(name="rows2", bufs=2))
        with tc.tile_pool(name="ptmp", bufs=3) as ptmp, \
             tc.tile_pool(name="ln2w", bufs=2) as ln2w:
            # proj in two stages: partial chains over pairs 0..4 can start
            # while the last pair's normalize is still in flight; the t=5
            # matmul + epilogue + LN2 stats follow per output chunk.
            Ptiles = {}

            def proj_A(co):
                P = psP.tile([128, NO], F32, tag="S", name="P")
                Ptiles[co] = P
                for t in range(CC - 1):
                    lhsT = wproj_sb[:, t, co * 128:(co + 1) * 128]
                    for s in range(NO // 512):
                        sl = slice(s * 512, (s + 1) * 512)
                        nc.tensor.matmul(P[:, sl], lhsT, oTpair[:, t, sl],
                                         start=(t == 0), stop=False)

            musum2 = ps_st2.tile([1, NO], F32, tag="oT", name="musum2")
            sqsum2 = ps_st2.tile([1, NO], F32, tag="oT", name="sqsum2")

            def proj_B(co):
                P = Ptiles.pop(co)
                lhsT = wproj_sb[:, CC - 1, co * 128:(co + 1) * 128]
                for s in range(NO // 512):
                    sl = slice(s * 512, (s + 1) * 512)
                    nc.tensor.matmul(P[:, sl], lhsT, oTpair[:, CC - 1, sl],
                                     start=False, stop=True)
                tmp = ptmp.tile([128, NO], F32, tag="ptmp", name="tmp")
                nc.scalar.activation(tmp, P, AF.Identity, bias=bproj_sb[:, co, 0:1])
                nc.vector.tensor_add(x2T[:, co, :], tmp, xTf[:, co, :])
                # LN2 stats for this chunk
                x2b = ln2w.tile([128, NO], BF, tag="x2b")
                nc.vector.tensor_copy(x2b, x2T[:, co, :])
                sq2 = ln2w.tile([128, NO], BF, tag="sq2")
                nc.vector.tensor_mul(sq2, x2T[:, co, :], x2T[:, co, :])
                for s in range(NO // 512):
                    sl = slice(s * 512, (s + 1) * 512)
                    nc.tensor.matmul(musum2[:, sl], ones_sb[:, 0:1], x2b[:, sl],
                                     start=(co == 0), stop=(co == CC - 1))
                    nc.tensor.matmul(sqsum2[:, sl], ones_sb[:, 0:1], sq2[:, sl],
                                     start=(co == 0), stop=(co == CC - 1))

            for op in (lambda: proj_A(0), lambda: proj_A(1), lambda: proj_B(0),
                       lambda: proj_A(2), lambda: proj_B(1), lambda: proj_A(3),
                       lambda: proj_B(2), lambda: proj_A(4), lambda: proj_B(3),
                       lambda: proj_A(5), lambda: proj_B(4), lambda: proj_B(5)):
                op()

            mu2_bf = rows2.tile([1, NO], BF, tag="mubf", bufs=1)
            rs2_bf = rows2.tile([1, NO], BF, tag="rsbf", bufs=1)
            _ln_rows(nc, rows2, musum2, sqsum2, mu2_bf, rs2_bf, epst, NO)
        es_attps.close()
        with ln2stack:
            with tc.tile_pool(name="ps_bc2", bufs=1, space="PSUM") as ps_bc, \
                 tc.tile_pool(name="ln2tmp", bufs=2) as ln2tmp:
                muB2 = ps_bc.tile([128, NO], F32)
                rsB2 = ps_bc.tile([128, NO], F32)
                for s in range(NO // 512):
                    sl = slice(s * 512, (s + 1) * 512)
                    nc.tensor.matmul(muB2[:, sl], ones_sb[0:1, :], mu2_bf[:, sl])
                    nc.tensor.matmul(rsB2[:, sl], ones_sb[0:1, :], rs2_bf[:, sl])
                for cc in range(CC):
                    t1 = ln2tmp.tile([128, NO], BF, tag="t", name="t1")
                    nc.vector.tensor_sub(t1, x2T[:, cc, :], muB2)
                    t2 = ln2tmp.tile([128, NO], BF, tag="t", name="t2")
                    nc.vector.tensor_mul(t2, t1, rsB2)
                    nc.scalar.activation(h2T[:, cc, :], t2, AF.Identity,
                                         bias=gb2_sb[:, cc, 1:2],
                                         scale=gb2_sb[:, cc, 0:1])

        # ---------- Phase 6: fc1 + gelu ----------
        pool_g2 = es.enter_context(tc.tile_pool(name="g2", bufs=1))
        g2T = pool_g2.tile([128, HC, NO], BF)

        with tc.tile_pool(name="ps_F", bufs=4, space="PSUM") as psF:
            for hc in range(HC):
                F = psF.tile([128, NO], F32, tag="F", name="F")
                for cc in range(CC):
                    lhsT = wfc1_sb[:, cc, hc * 128:(hc + 1) * 128]
                    for s in range(NO // 512):
                        sl = slice(s * 512, (s + 1) * 512)
                        nc.tensor.matmul(F[:, sl], lhsT, h2T[:, cc, sl],
                                         start=(cc == 0), stop=(cc == CC - 1))
                nc.scalar.activation(g2T[:, hc, :], F, AF.Gelu,
                                     bias=bfc1_sb[:, hc, 0:1])

        # ---------- Phase 7: fc2 + residual + out ----------
        with tc.tile_pool(name="ps_O", bufs=4, space="PSUM") as psOu, \
             tc.tile_pool(name="otmp", bufs=2) as otmp:
            for co in range(CC):
                O = psOu.tile([128, NO], F32, tag="O", name="O")
                for hc in range(HC):
                    lhsT = wfc2_sb[:, hc, co * 128:(co + 1) * 128]
                    for s in range(NO // 512):
                        sl = slice(s * 512, (s + 1) * 512)
                        nc.tensor.matmul(O[:, sl], lhsT, g2T[:, hc, sl],
                                         start=(hc == 0), stop=(hc == HC - 1))
                outt = otmp.tile([128, NO], F32, tag="outt", name="outt")
                nc.vector.tensor_add(outt, O, x2T[:, co, :])
                nc.vector.tensor_scalar_add(outt, outt, bfc2_sb[:, co, 0:1])
                nc.sync.dma_start(
                    out=dt_["outT"].ap().rearrange("(t p) n -> p t n", p=128)[:, co, :],
                    in_=outt)


def _prep_core_inputs(c, x, w_qkv, w_proj, b_proj, ln1_g, ln1_b, ln2_g, ln2_b,
                      w_fc1, b_fc1, w_fc2, b_fc2):
    b, half = c // 2, c % 2
    own = slice(half * NO, (half + 1) * NO)
    other = slice((1 - half) * NO, (2 - half) * NO)
    xb = x[b]                                   # [2048, 768] fp32
    xperm = np.concatenate([xb[own], xb[other]], axis=0)   # [2048, 768]
    xTb = np.ascontiguousarray(xperm.T).astype(BF_NP)      # [768, 2048] bf16
    xTf = np.ascontiguousarray(xb[own].T)                  # [768, 1024] fp32
    gb1 = np.stack([ln1_g, ln1_b], axis=1).astype(np.float32)  # [768, 2]
    gb2 = np.stack([ln2_g, ln2_b], axis=1).astype(np.float32)
    return {
        "xTb": xTb,
        "xTf": xTf,
        "wqkv": w_qkv.astype(BF_NP),
        "wproj": w_proj.astype(BF_NP),
        "wfc1": w_fc1.astype(BF_NP),
        "wfc2": w_fc2.astype(BF_NP),
        "gb1": gb1,
        "gb2": gb2,
        "bproj": b_proj.reshape(DIM, 1).astype(np.float32),
        "bfc1": b_fc1.reshape(HID, 1).astype(np.float32),
        "bfc2": b_fc2.reshape(DIM, 1).astype(np.float32),
        "ones": np.ones((128, 128), dtype=BF_NP),
    }


def kernel(x, ln1_g, ln1_b, w_qkv, w_proj, b_proj, ln2_g, ln2_b,
           w_fc1, b_fc1, w_fc2, b_fc2, _trace=False, _tmpdir=None):
    x = np.asarray(x, dtype=np.float32)
    args = [np.asarray(a, dtype=np.float32) for a in
            (w_qkv, w_proj, b_proj, ln1_g, ln1_b, ln2_g, ln2_b,
             w_fc1, b_fc1, w_fc2, b_fc2)]
    if _compiled[0] is None:
        _compiled[0] = _build_nc()
    nc = _compiled[0]
    in_maps = [_prep_core_inputs(c, x, *args) for c in range(8)]
    res = run_bass_kernel_spmd(nc, in_maps, list(range(8)),
                               trace=_trace, tmpdir=_tmpdir)
    out = np.empty((B, N, DIM), dtype=np.float32)
    for c in range(8):
        b, half = c // 2, c % 2
        out[b, half * NO:(half + 1) * NO, :] = res.results[c]["outT"].T
    kernel._last_results = res
    return out



# revision 15
# speedup vs baseline: 1.1987x; 1.1987x over previous
"""Trainium2 Bass kernel for a dense transformer block (nn_Block_15058155340426).

Full (unsharded) inputs -> full output.  Internally: 8-core SPMD, no
cross-core communication.  Core c handles batch b=c//2, token half c%2.
Each core redundantly computes LN1 + K/V for its batch's full 2048 tokens
(order: [own 1024 | other 1024]) and runs attention/proj/MLP for its own
1024 query tokens.  All on-chip activations use the transposed [feature,
token] layout; the residual stream stays fp32.

Perf structure:
 - The qkv / fc1 / fc2 GEMMs run in fp8e4 with DoubleRow perf mode
   (2 fp8 weights per PE cell -> ~1.7x streaming over bf16).  LN outputs
   (hT, h2T) and gelu output (g2T) are written as fp8 directly by ACT.
 - Attention S matmuls (K=64) are row-tiled pairs (partitions 0:64 /
   64:128) that execute concurrently on the PE; S/O stay bf16.
 - K/Q projection GEMMs are interleaved into the attention stream as PE
   filler; the phase is paced by the ACT exp throughput.
 - LN mean/rstd broadcasts are cast once to bf16 SBUF so the per-chunk
   normalize ops hit the DVE 4x fast path.
 - The x input loads as 12 half-chunk DMAs spread over 4 engine queues.
"""

import numpy as np
import ml_dtypes

import concourse.bass as bass
import concourse.bacc as bacc
import concourse.tile as tile
from concourse import mybir
from concourse.bass_utils import run_bass_kernel_spmd

BF = mybir.dt.bfloat16
F32 = mybir.dt.float32
F8 = mybir.dt.float8e4
BF_NP = ml_dtypes.bfloat16
F8_NP = ml_dtypes.float8_e4m3

DIM = 768
HEADS = 12
HD = 64          # head dim
HID = 3072
B = 4
N = 2048         # tokens per batch (full)
NO = 1024        # own tokens per core
CC = DIM // 128  # 6 feature chunks
TP = CC // 2     # 3 fp8 DoubleRow chunk-pairs
HC = HID // 128  # 24 hidden chunks
MC = N // 128    # 16 key chunks
EPS = 1e-5
SCALE = HD ** -0.5
WS = 64.0        # fp8 weight pre-scale: w~N(0,0.02) sits in e4m3's subnormal
                 # range; x64 moves it into normals (6% rel quantization)

AF = mybir.ActivationFunctionType
DR = mybir.MatmulPerfMode.DoubleRow

_compiled = [None]


def _build_nc():
    nc = bacc.Bacc("TRN2", target_bir_lowering=False, debug=False, num_devices=8)

    # ---- DRAM I/O ----
    dt_ = {}
    dt_["xTb"] = nc.dram_tensor("xTb", [DIM, N], BF, kind="ExternalInput")
    dt_["xTf"] = nc.dram_tensor("xTf", [DIM, NO], F32, kind="ExternalInput")
    dt_["wqkv"] = nc.dram_tensor("wqkv", [DIM, 3 * DIM], F8, kind="ExternalInput")
    dt_["wproj"] = nc.dram_tensor("wproj", [DIM, DIM], BF, kind="ExternalInput")
    dt_["wfc1"] = nc.dram_tensor("wfc1", [DIM, HID], F8, kind="ExternalInput")
    dt_["wfc2"] = nc.dram_tensor("wfc2", [HID, DIM], F8, kind="ExternalInput")
    dt_["gb1"] = nc.dram_tensor("gb1", [DIM, 2], F32, kind="ExternalInput")
    dt_["gb2"] = nc.dram_tensor("gb2", [DIM, 2], F32, kind="ExternalInput")
    dt_["bproj"] = nc.dram_tensor("bproj", [DIM, 1], F32, kind="ExternalInput")
    dt_["bfc1"] = nc.dram_tensor("bfc1", [HID, 1], F32, kind="ExternalInput")
    dt_["bfc2"] = nc.dram_tensor("bfc2", [DIM, 1], F32, kind="ExternalInput")
    dt_["ones"] = nc.dram_tensor("ones", [128, 128], BF, kind="ExternalInput")
    dt_["outT"] = nc.dram_tensor("outT", [DIM, NO], F32, kind="ExternalOutput")

    with tile.TileContext(nc, pool_alloc_mode="queue") as tc:
        _emit(nc, tc, dt_)
    nc.compile()
    return nc


def _ln_rows(nc, rows, musum, sqsum, mu_bf, rs_bf, epst, n):
    """Row math on partition 0: mean/var -> bf16 mu and rsqrt rows."""
    mu_f = rows.tile([1, n], F32, tag="rowf", name="mu_f")
    nc.scalar.mul(mu_f, musum, 1.0 / DIM)
    nc.vector.tensor_copy(mu_bf, mu_f)
    var = rows.tile([1, n], F32, tag="rowf", name="var")
    nc.vector.tensor_mul(var, mu_f, mu_f)          # var := mu^2
    ex2 = rows.tile([1, n], F32, tag="rowf", name="ex2")
    nc.scalar.mul(ex2, sqsum, 1.0 / DIM)
    nc.vector.tensor_sub(var, ex2, var)            # var := E[x^2] - mu^2
    sd = rows.tile([1, n], F32, tag="rowf", name="sd")
    nc.scalar.activation(sd, var, AF.Sqrt, bias=epst[0:1, 0:1])
    rs_f = rows.tile([1, n], F32, tag="rowf", name="rs_f")
    nc.vector.reciprocal_approx_fast(rs_f, sd)
    nc.vector.tensor_copy(rs_bf, rs_f)


def _emit(nc, tc, dt_):
    from contextlib import ExitStack
    es = ExitStack()            # whole-kernel pools (released at end, LIFO)
    es_qkv = ExitStack()        # hT + wqkv: until attention done
    es_kqv = ExitStack()        # kTs/qTs/vt: until attention done
    es_att = ExitStack()        # attention working tiles
    with es:
        # ---------- pool allocation (stack order = reverse of release) ----------
        const = es.enter_context(tc.tile_pool(name="const", bufs=1))
        ones_sb = const.tile([128, 128], BF)
        gb1_sb = const.tile([128, CC, 2], F32)
        gb2_sb = const.tile([128, CC, 2], F32)
        bproj_sb = const.tile([128, CC, 1], F32)
        bfc1_sb = const.tile([128, HC, 1], F32)
        bfc2_sb = const.tile([128, CC, 1], F32)
        epst = const.tile([1, 1], F32)

        # long-lived: residual (becomes x2T in place), attention output, wproj
        resid = es.enter_context(tc.tile_pool(name="resid", bufs=1))
        xTf = resid.tile([128, CC, NO], F32)
        x2T = xTf  # alias: P4 writes the residual sum back into this tile
        oTpair = resid.tile([128, CC, NO], BF)
        wproj_sb = resid.tile([128, CC, DIM], BF)

        pool_h = es_qkv.enter_context(tc.tile_pool(name="h", bufs=1))
        hT = pool_h.tile([128, CC, N], F8)
        pool_wqkv = es_qkv.enter_context(tc.tile_pool(name="wqkv", bufs=1))
        wqkv_sb = pool_wqkv.tile([128, CC, 3 * DIM], F8)

        pool_kqv = es_kqv.enter_context(tc.tile_pool(name="kqv", bufs=1))
        kTs = pool_kqv.tile([128, CC, N], BF)       # head pairs: part 0:64 = head 2hp
        qTs = pool_kqv.tile([128, CC, NO], BF)
        vt = pool_kqv.tile([128, MC, HEADS, HD + 1], BF)

        # ---------- Phase 1+2: LN1 (two pipelined halves) + V GEMMs ----------
        with tc.tile_pool(name="ln1", bufs=1) as ln1, \
             tc.tile_pool(name="rows1", bufs=2) as rows1, \
             tc.tile_pool(name="ln1sq", bufs=2) as ln1sq, \
             tc.tile_pool(name="ln1bc", bufs=2) as ln1bc, \
             tc.tile_pool(name="ln1tmp", bufs=2) as ln1tmp, \
             tc.tile_pool(name="ps_ln1", bufs=1, space="PSUM") as psL:
            xTb = ln1.tile([128, CC, N], BF)
            # DMA priority: xTb half 0 (gates everything) first, spread over
            # 2 engine queues; wqkv rides its own (gpsimd) queue in parallel
            dma_engines = [nc.sync, nc.scalar]
            xsrc = dt_["xTb"].ap().rearrange("(t p) n -> p t n", p=128)
            nc.gpsimd.dma_start(out=wqkv_sb[:], in_=dt_["wqkv"].ap().rearrange("(t p) m -> p t m", p=128))
            for half in range(2):
                hsl = slice(half * NO, (half + 1) * NO)
                for cc in range(CC):
                    eng = dma_engines[cc % 2]
                    eng.dma_start(out=xTb[:, cc, hsl], in_=xsrc[:, cc, hsl])
            nc.sync.dma_start(out=ones_sb[:], in_=dt_["ones"][:])
            nc.sync.dma_start(out=gb1_sb[:], in_=dt_["gb1"].ap().rearrange("(t p) k -> p t k", p=128))
            nc.sync.dma_start(out=gb2_sb[:], in_=dt_["gb2"].ap().rearrange("(t p) k -> p t k", p=128))
            nc.sync.dma_start(out=bproj_sb[:], in_=dt_["bproj"].ap().rearrange("(t p) k -> p t k", p=128))
            nc.sync.dma_start(out=bfc1_sb[:], in_=dt_["bfc1"].ap().rearrange("(t p) k -> p t k", p=128))
            nc.sync.dma_start(out=bfc2_sb[:], in_=dt_["bfc2"].ap().rearrange("(t p) k -> p t k", p=128))
            nc.vector.memset(epst, EPS)
            nc.vector.memset(vt, 1.0)

            for half in range(2):
                nsl = slice(half * NO, (half + 1) * NO)
                st = psL.tile([1, 2 * NO], F32, tag="st", bufs=1, name="st")
                for cc in range(CC):
                    sq = ln1sq.tile([128, NO], BF, tag="sq")
                    nc.vector.tensor_mul(sq, xTb[:, cc, nsl], xTb[:, cc, nsl])
                    for s in range(NO // 512):
                        isl = slice(s * 512, (s + 1) * 512)
                        xsl = slice(half * NO + s * 512, half * NO + (s + 1) * 512)
                        nc.tensor.matmul(st[:, isl], ones_sb[:, 0:1], xTb[:, cc, xsl],
                                         start=(cc == 0), stop=(cc == CC - 1))
                        nc.tensor.matmul(st[:, NO + s * 512: NO + (s + 1) * 512],
                                         ones_sb[:, 0:1], sq[:, isl],
                                         start=(cc == 0), stop=(cc == CC - 1))
                mu_bf = rows1.tile([1, NO], BF, tag="mubf", bufs=2, name="mu_bf")
                rs_bf = rows1.tile([1, NO], BF, tag="rsbf", bufs=2, name="rs_bf")
                _ln_rows(nc, rows1, st[:, 0:NO], st[:, NO:2 * NO], mu_bf, rs_bf,
                         epst, NO)
                muB = psL.tile([128, NO], F32, tag="bc", bufs=2, name="muB")
                rsB = psL.tile([128, NO], F32, tag="bc", bufs=2, name="rsB")
                for s in range(NO // 512):
                    isl = slice(s * 512, (s + 1) * 512)
                    nc.tensor.matmul(muB[:, isl], ones_sb[0:1, :], mu_bf[:, isl])
                    nc.tensor.matmul(rsB[:, isl], ones_sb[0:1, :], rs_bf[:, isl])
                # cast the broadcasts once: the per-chunk normalize ops then
                # run bf16-only SBUF->SBUF (DVE 4x fast path)
                muBb = ln1bc.tile([128, NO], BF, tag="mu", bufs=2, name="muBb")
                rsBb = ln1bc.tile([128, NO], BF, tag="rs", bufs=2, name="rsBb")
                nc.vector.tensor_copy(muBb, muB)
                nc.vector.tensor_copy(rsBb, rsB)
                for cc in range(CC):
                    t1 = ln1tmp.tile([128, NO], BF, tag="t", name="t1")
                    nc.vector.tensor_sub(t1, xTb[:, cc, nsl], muBb)
                    t2 = ln1tmp.tile([128, NO], BF, tag="t", name="t2")
                    nc.vector.tensor_mul(t2, t1, rsBb)
                    nc.scalar.activation(hT[:, cc, nsl], t2, AF.Identity,
                                         bias=gb1_sb[:, cc, 1:2],
                                         scale=gb1_sb[:, cc, 0:1])
                # V GEMMs for this half's key chunks (fp8 DoubleRow)
                for mi in range(half * (MC // 2), (half + 1) * (MC // 2)):
                    vps = psL.tile([128, DIM], F32, tag="bc", bufs=2, name="vps")
                    for t in range(TP):
                        lhsT = hT[:, 2 * t:2 * t + 2, mi * 128: (mi + 1) * 128]
                        nc.tensor.matmul(vps[:, 0:512], lhsT,
                                         wqkv_sb[:, 2 * t:2 * t + 2, 1536:2048],
                                         start=(t == 0), stop=(t == TP - 1),
                                         perf_mode=DR)
                        nc.tensor.matmul(vps[:, 512:768], lhsT,
                                         wqkv_sb[:, 2 * t:2 * t + 2, 2048:2304],
                                         start=(t == 0), stop=(t == TP - 1),
                                         perf_mode=DR)
                    nc.vector.tensor_copy(
                        vt[:, mi, :, 0:HD],
                        vps.rearrange("p (h d) -> p h d", h=HEADS))
                # K/Q for pair 0, this token-half (enables early attention)
                for col in range(half * 2, half * 2 + 2):
                    kps = psL.tile([128, 512], F32, tag="bc", bufs=2, name="kps")
                    for t in range(TP):
                        nc.tensor.matmul(
                            kps[:, 0:512],
                            wqkv_sb[:, 2 * t:2 * t + 2, DIM: DIM + 128],
                            hT[:, 2 * t:2 * t + 2, col * 512:(col + 1) * 512],
                            start=(t == 0), stop=(t == TP - 1), perf_mode=DR)
                    nc.vector.tensor_copy(kTs[:, 0, col * 512:(col + 1) * 512],
                                          kps[:, 0:512])
                qps = psL.tile([128, 512], F32, tag="bc", bufs=2, name="qps")
                for t in range(TP):
                    nc.tensor.matmul(
                        qps[:, 0:512], wqkv_sb[:, 2 * t:2 * t + 2, 0:128],
                        hT[:, 2 * t:2 * t + 2, half * 512:(half + 1) * 512],
                        start=(t == 0), stop=(t == TP - 1), perf_mode=DR)
                nc.vector.tensor_copy(qTs[:, 0, half * 512:(half + 1) * 512],
                                      qps[:, 0:512])

        # ---------- Phase 3: attention with K/Q GEMMs interleaved ----------
        es_attps = ExitStack()
        psS = es_attps.enter_context(tc.tile_pool(name="ps_S", bufs=2, space="PSUM"))
        psO = es_attps.enter_context(tc.tile_pool(name="ps_oT", bufs=2, space="PSUM"))
        attw = es_att.enter_context(tc.tile_pool(name="attw", bufs=3))
        atodd = es_att.enter_context(tc.tile_pool(name="atodd", bufs=1))

        def kq_chunks(hp):
            """Six PE filler chunks producing kTs / qTs for pair hp; each is
            a closure so it can be spliced between attention m-steps."""
            out = []
            for half in range(4):   # k: 4 x 512 columns
                def k_grp(hp=hp, half=half):
                    kps = psS.tile([128, 512], F32, tag="S", name="kps")
                    for t in range(TP):
                        nc.tensor.matmul(
                            kps[:],
                            wqkv_sb[:, 2 * t:2 * t + 2, DIM + hp * 128: DIM + (hp + 1) * 128],
                            hT[:, 2 * t:2 * t + 2, half * 512:(half + 1) * 512],
                            start=(t == 0), stop=(t == TP - 1), perf_mode=DR)
                    nc.vector.tensor_copy(kTs[:, hp, half * 512:(half + 1) * 512], kps)
                out.append(k_grp)
            for half in range(2):   # q: 2 x 512 columns (own tokens)
                def q_grp(hp=hp, half=half):
                    qps = psS.tile([128, 512], F32, tag="S", name="qps")
                    for t in range(TP):
                        nc.tensor.matmul(
                            qps[:], wqkv_sb[:, 2 * t:2 * t + 2, hp * 128: (hp + 1) * 128],
                            hT[:, 2 * t:2 * t + 2, half * 512:(half + 1) * 512],
                            start=(t == 0), stop=(t == TP - 1), perf_mode=DR)
                    nc.vector.tensor_copy(qTs[:, hp, half * 512:(half + 1) * 512], qps)
                out.append(q_grp)
            return out

        def stage(oTp):
            # one cast drains the whole unnormalized head (incl colsum row)
            # to SBUF so the PSUM banks free immediately for the next pair
            oTu = attw.tile([65, NO], F32, tag="oTu", bufs=2, name="oTu")
            nc.vector.tensor_copy(oTu, oTp)
            return oTu

        def normalize(hp, oTu0, oTu1):
            # full-tile fast reciprocal of the staged head (only the colsum
            # row 64 is meaningful; a row-slice approx_fast is broken) ->
            # broadcast row 64 to 64 partitions -> scale the head.
            for off, oTu in ((0, oTu0), (64, oTu1)):
                csr = attw.tile([65, NO], F32, tag="csrf", bufs=2, name="csr")
                nc.vector.reciprocal_approx_fast(csr, oTu)
                csrb = attw.tile([65, NO], BF, tag="csrb", bufs=2, name="csrb")
                nc.vector.tensor_copy(csrb[64:65, :], csr[64:65, :])
                rcpB = psS.tile([64, NO], F32, tag="S", name="rcpB")
                for s in range(NO // 512):
                    sl = slice(s * 512, (s + 1) * 512)
                    nc.tensor.matmul(rcpB[:, sl], ones_sb[64:65, 0:64],
                                     csrb[64:65, sl])
                rcpS = attw.tile([64, NO], BF, tag="rcps", bufs=2, name="rcpS")
                nc.vector.tensor_copy(rcpS, rcpB)
                if off == 0:
                    nc.vector.tensor_mul(oTpair[0:64, hp, :], oTu[0:64, :], rcpS)
                else:
                    stag = atodd.tile([64, NO], BF, tag="stag", name="stag")
                    nc.vector.tensor_mul(stag, oTu[0:64, :], rcpS)
                    nc.sync.dma_start(out=oTpair[64:128, hp, :], in_=stag)

        pending = None  # (hp, oTu0, oTu1) deferred so PE never waits on recip
        for hp in range(CC):
            h0, h1 = 2 * hp, 2 * hp + 1
            filler = kq_chunks(hp + 1) if hp + 1 < CC else []
            if hp == 3:
                # load late-phase inputs during attention (DMA engines idle)
                nc.sync.dma_start(out=wproj_sb[:],
                                  in_=dt_["wproj"].ap().rearrange("(t p) m -> p t m", p=128))
                nc.sync.dma_start(out=xTf[:],
                                  in_=dt_["xTf"].ap().rearrange("(t p) n -> p t n", p=128))
            oTp0 = psO.tile([65, NO], F32, tag="oT", name="oTp0")
            oTp1 = psO.tile([65, NO], F32, tag="oT", name="oTp1")
            deferred = []   # avs of mi==0 held back so the new pair opens
                            # with a dense run of S matmuls
            for mi in range(MC):
                ats = []
                for off in (0, 64):
                    S = psS.tile([128, NO], F32, tag="S", name="S")
                    for s in range(NO // 512):
                        sl = slice(s * 512, (s + 1) * 512)
                        nc.tensor.matmul(S[:, sl],
                                         kTs[off:off + 64, hp, mi * 128:(mi + 1) * 128],
                                         qTs[off:off + 64, hp, sl])
                    at = attw.tile([128, NO], BF, tag="at", name="at", bufs=4)
                    # q and k each carry the x{WS} weight scale -> /WS^2
                    nc.scalar.activation(at, S, AF.Exp, scale=SCALE / (WS * WS))
                    ats.append(at)
                # splice one K/Q filler chunk for the NEXT pair here: keeps
                # the PE busy while ACT computes the exps
                if mi in (1, 3, 6, 9, 12, 14) and filler:
                    filler.pop(0)()
                elif not filler and mi % 2 == 0:
                    # no real filler (last pair): dummy weight loads keep the
                    # PE array active so the HAM clock gate stays open
                    nc.tensor.ldweights(weights=vt[:, mi, 0, :])
                    nc.tensor.ldweights(weights=vt[:, mi, 6, :])
                work = [(h0, oTp0, ats[0]), (h1, oTp1, ats[1])]
                if mi == 0:
                    deferred = work
                    continue
                for h, oTp, at in (deferred + work if mi == 1 else work):
                    for s in range(NO // 512):
                        sl = slice(s * 512, (s + 1) * 512)
                        nc.tensor.matmul(oTp[:, sl], vt[:, mi if at in
                                         (ats[0], ats[1]) else 0, h, :], at[:, sl],
                                         start=(at not in (ats[0], ats[1])),
                                         stop=(mi == MC - 1))
                if mi == 1:
                    deferred = []
                if mi == 2 and pending is not None:
                    normalize(*pending)
                    pending = None
            for grp in filler:
                grp()
            oTu0, oTu1 = stage(oTp0), stage(oTp1)
            pending = (hp, oTu0, oTu1)
        normalize(*pending)
        es_att.close()
        es_kqv.close()
        es_qkv.close()
        psP = psS    # proj accumulators reuse the attention score slots
        ps_st2 = psO  # LN2 stats reuse the attention output slots

        # ---------- Phase 4: proj + residual (written in place into xTf) ----------
        pool_wfc1 = es.enter_context(tc.tile_pool(name="wfc1", bufs=1))
        wfc1_sb = pool_wfc1.tile([128, CC, HID], F8)
        for cc in range(CC):
            nc.sync.dma_start(
                out=wfc1_sb[:, cc, :],
                in_=dt_["wfc1"].ap().rearrange("(t p) m -> p t m", p=128)[:, cc, :])
        pool_wfc2 = es.enter_context(tc.tile_pool(name="wfc2", bufs=1))
        wfc2_sb = pool_wfc2.tile([128, HC, DIM], F8)
        for hc2 in range(0, HC, 6):
            nc.sync.dma_start(
                out=wfc2_sb[:, hc2:hc2 + 6, :],
                in_=dt_["wfc2"].ap().rearrange("(t p) m -> p t m", p=128)[:, hc2:hc2 + 6, :])
        pool_h2 = es.enter_context(tc.tile_pool(name="h2", bufs=1))
        h2T = pool_h2.tile([128, CC, NO], F8)
        ln2stack = ExitStack()
        rows2 = ln2stack.enter_context(tc.tile_pool(name="rows2", bufs=2))
        x2bp = ln2stack.enter_context(tc.tile_pool(name="x2bp", bufs=1))
        x2b16 = x2bp.tile([128, CC, NO], BF)   # bf16 copy of x2, reused by LN2
        with tc.tile_pool(name="ptmp", bufs=3) as ptmp, \
             tc.tile_pool(name="ln2w", bufs=2) as ln2w:
            # proj in two stages: partial chains over pairs 0..4 can start
            # while the last pair's normalize is still in flight; the t=5
            # matmul + epilogue + LN2 stats follow per output chunk.
            Ptiles = {}

            def proj_A(co):
                P = psP.tile([128, NO], F32, tag="S", name="P")
                Ptiles[co] = P
                for t in range(CC - 1):
                    lhsT = wproj_sb[:, t, co * 128:(co + 1) * 128]
                    for s in range(NO // 512):
                        sl = slice(s * 512, (s + 1) * 512)
                        nc.tensor.matmul(P[:, sl], lhsT, oTpair[:, t, sl],
                                         start=(t == 0), stop=False)

            musum2 = ps_st2.tile([1, NO], F32, tag="oT", name="musum2")
            sqsum2 = ps_st2.tile([1, NO], F32, tag="oT", name="sqsum2")

            def proj_B(co):
                P = Ptiles.pop(co)
                lhsT = wproj_sb[:, CC - 1, co * 128:(co + 1) * 128]
                for s in range(NO // 512):
                    sl = slice(s * 512, (s + 1) * 512)
                    nc.tensor.matmul(P[:, sl], lhsT, oTpair[:, CC - 1, sl],
                                     start=False, stop=True)
                tmp = ptmp.tile([128, NO], F32, tag="ptmp", name="tmp")
                # oTpair carries the x{WS} v-scale -> undo it here
                nc.scalar.activation(tmp, P, AF.Identity,
                                     bias=bproj_sb[:, co, 0:1], scale=1.0 / WS)
                nc.vector.tensor_add(x2T[:, co, :], tmp, xTf[:, co, :])
                # LN2 stats for this chunk (bf16 copy kept for the LN2 apply)
                nc.vector.tensor_copy(x2b16[:, co, :], x2T[:, co, :])
                sq2 = ln2w.tile([128, NO], BF, tag="sq2")
                nc.vector.tensor_mul(sq2, x2T[:, co, :], x2T[:, co, :])
                for s in range(NO // 512):
                    sl = slice(s * 512, (s + 1) * 512)
                    nc.tensor.matmul(musum2[:, sl], ones_sb[:, 0:1],
                                     x2b16[:, co, sl],
                                     start=(co == 0), stop=(co == CC - 1))
                    nc.tensor.matmul(sqsum2[:, sl], ones_sb[:, 0:1], sq2[:, sl],
                                     start=(co == 0), stop=(co == CC - 1))

            for op in (lambda: proj_A(0), lambda: proj_A(1), lambda: proj_B(0),
                       lambda: proj_A(2), lambda: proj_B(1), lambda: proj_A(3),
                       lambda: proj_B(2), lambda: proj_A(4), lambda: proj_B(3),
                       lambda: proj_A(5), lambda: proj_B(4), lambda: proj_B(5)):
                op()

            mu2_bf = rows2.tile([1, NO], BF, tag="mubf", bufs=1)
            rs2_bf = rows2.tile([1, NO], BF, tag="rsbf", bufs=1)
            _ln_rows(nc, rows2, musum2, sqsum2, mu2_bf, rs2_bf, epst, NO)
        es_attps.close()
        with ln2stack:
            with tc.tile_pool(name="ps_bc2", bufs=1, space="PSUM") as ps_bc, \
                 tc.tile_pool(name="ln2bc", bufs=1) as ln2bc, \
                 tc.tile_pool(name="ln2tmp", bufs=2) as ln2tmp:
                muB2 = ps_bc.tile([128, NO], F32)
                rsB2 = ps_bc.tile([128, NO], F32)
                for s in range(NO // 512):
                    sl = slice(s * 512, (s + 1) * 512)
                    nc.tensor.matmul(muB2[:, sl], ones_sb[0:1, :], mu2_bf[:, sl])
                    nc.tensor.matmul(rsB2[:, sl], ones_sb[0:1, :], rs2_bf[:, sl])
                muB2b = ln2bc.tile([128, NO], BF)
                rsB2b = ln2bc.tile([128, NO], BF)
                nc.vector.tensor_copy(muB2b, muB2)
                nc.vector.tensor_copy(rsB2b, rsB2)
                for cc in range(CC):
                    t1 = ln2tmp.tile([128, NO], BF, tag="t", name="t1")
                    nc.vector.tensor_sub(t1, x2b16[:, cc, :], muB2b)
                    t2 = ln2tmp.tile([128, NO], BF, tag="t", name="t2")
                    nc.vector.tensor_mul(t2, t1, rsB2b)
                    nc.scalar.activation(h2T[:, cc, :], t2, AF.Identity,
                                         bias=gb2_sb[:, cc, 1:2],
                                         scale=gb2_sb[:, cc, 0:1])

        # ---------- Phase 6: fc1 + gelu (fp8 DoubleRow) ----------
        pool_g2 = es.enter_context(tc.tile_pool(name="g2", bufs=1))
        g2T = pool_g2.tile([128, HC, NO], F8)

        with tc.tile_pool(name="ps_F", bufs=4, space="PSUM") as psF:
            for hc in range(HC):
                F = psF.tile([128, NO], F32, tag="F", name="F")
                for t in range(TP):
                    lhsT = wfc1_sb[:, 2 * t:2 * t + 2, hc * 128:(hc + 1) * 128]
                    for s in range(NO // 512):
                        sl = slice(s * 512, (s + 1) * 512)
                        nc.tensor.matmul(F[:, sl], lhsT, h2T[:, 2 * t:2 * t + 2, sl],
                                         start=(t == 0), stop=(t == TP - 1),
                                         perf_mode=DR)
                nc.scalar.activation(g2T[:, hc, :], F, AF.Gelu,
                                     bias=bfc1_sb[:, hc, 0:1], scale=1.0 / WS)

        # ---------- Phase 7: fc2 + residual + out (fp8 DoubleRow) ----------
        with tc.tile_pool(name="ps_O", bufs=4, space="PSUM") as psOu, \
             tc.tile_pool(name="otmp", bufs=2) as otmp:
            for co in range(CC):
                O = psOu.tile([128, NO], F32, tag="O", name="O")
                for j in range(HC // 2):
                    lhsT = wfc2_sb[:, 2 * j:2 * j + 2, co * 128:(co + 1) * 128]
                    for s in range(NO // 512):
                        sl = slice(s * 512, (s + 1) * 512)
                        nc.tensor.matmul(O[:, sl], lhsT, g2T[:, 2 * j:2 * j + 2, sl],
                                         start=(j == 0), stop=(j == HC // 2 - 1),
                                         perf_mode=DR)
                outt = otmp.tile([128, NO], F32, tag="outt", name="outt")
                nc.vector.scalar_tensor_tensor(
                    outt, O, 1.0 / WS, x2T[:, co, :],
                    op0=mybir.AluOpType.mult, op1=mybir.AluOpType.add)
                nc.vector.tensor_scalar_add(outt, outt, bfc2_sb[:, co, 0:1])
                nc.sync.dma_start(
                    out=dt_["outT"].ap().rearrange("(t p) n -> p t n", p=128)[:, co, :],
                    in_=outt)


def _prep_core_inputs(c, x, w_qkv, w_proj, b_proj, ln1_g, ln1_b, ln2_g, ln2_b,
                      w_fc1, b_fc1, w_fc2, b_fc2):
    b, half = c // 2, c % 2
    own = slice(half * NO, (half + 1) * NO)
    other = slice((1 - half) * NO, (2 - half) * NO)
    xb = x[b]                                   # [2048, 768] fp32
    xperm = np.concatenate([xb[own], xb[other]], axis=0)   # [2048, 768]
    xTb = np.ascontiguousarray(xperm.T).astype(BF_NP)      # [768, 2048] bf16
    xTf = np.ascontiguousarray(xb[own].T)                  # [768, 1024] fp32
    gb1 = np.stack([ln1_g, ln1_b], axis=1).astype(np.float32)  # [768, 2]
    gb2 = np.stack([ln2_g, ln2_b], axis=1).astype(np.float32)
    return {
        "xTb": xTb,
        "xTf": xTf,
        "wqkv": (w_qkv * WS).astype(F8_NP),
        "wproj": w_proj.astype(BF_NP),
        "wfc1": (w_fc1 * WS).astype(F8_NP),
        "wfc2": (w_fc2 * WS).astype(F8_NP),
        "gb1": gb1,
        "gb2": gb2,
        "bproj": b_proj.reshape(DIM, 1).astype(np.float32),
        "bfc1": b_fc1.reshape(HID, 1).astype(np.float32),
        "bfc2": b_fc2.reshape(DIM, 1).astype(np.float32),
        "ones": np.ones((128, 128), dtype=BF_NP),
    }


def kernel(x, ln1_g, ln1_b, w_qkv, w_proj, b_proj, ln2_g, ln2_b,
           w_fc1, b_fc1, w_fc2, b_fc2, _trace=False, _tmpdir=None):
    x = np.asarray(x, dtype=np.float32)
    args = [np.asarray(a, dtype=np.float32) for a in
            (w_qkv, w_proj, b_proj, ln1_g, ln1_b, ln2_g, ln2_b,
             w_fc1, b_fc1, w_fc2, b_fc2)]
    if _compiled[0] is None:
        _compiled[0] = _build_nc()
    nc = _compiled[0]
    in_maps = [_prep_core_inputs(c, x, *args) for c in range(8)]
    res = run_bass_kernel_spmd(nc, in_maps, list(range(8)),
                               trace=_trace, tmpdir=_tmpdir)
    out = np.empty((B, N, DIM), dtype=np.float32)
    for c in range(8):
        b, half = c // 2, c % 2
        out[b, half * NO:(half + 1) * NO, :] = res.results[c]["outT"].T
    kernel._last_results = res
    return out
